# revision 93
# baseline (speedup 1.0000x reference)
"""Trainium2 Bass kernel for Ernie4.5-VL attention (mRoPE + GQA causal attention).

Sharding: tensor-parallel over heads across 8 cores; each core computes 2 q
heads + its kv head and an o_proj row-slice partial; host sums 8 partials.

Layout/schedule (136.6us TimelineSim vs 206.7us for the f32 baseline):
- bf16 DMA I/O everywhere (xT, w, wo, yT partials): halves HBM traffic; rel
  err ~5e-3 vs the 2e-2 gate.
- few, batched DMAs (vs 120 in the baseline): constants packed into three
  tensors, output stores batched 2 hidden-tiles per transfer (each store
  launches right after its pair of psum->sbuf copies, keeping the drain
  tail short; 1-tile stores cost too much issue overhead, 4-tile batch too
  much tail latency).
- interleaved mRoPE via host-side even/odd weight-column permutation; the
  half-swap is two DVE cross-partition copies (no DMA, no PE); bf16 tables
  and operands get the 2x DVE rate.
- causal mask: rank-factored (mask_l.T @ mask_r[m]) additive -1e9 matmul
  accumulated onto the scores psum (an ACT->PSUM prefill + start=False
  accumulate silently loses the prefill on some banks on real HW - do not
  revive it). Diagonal blocks narrowed to their valid column range for
  scores/exp/AV/row-sums.
- row-sum: two chains (DVE odd-j + last, Pool even-j) merged in PSUM by a
  pair of accumulating ones-matmuls that also broadcast the denominator.
- software-pipelined PE schedule via a filler queue: chunk g's bank-major
  qkv projection (v,k first; psum->sbuf copy + rope emitted as queued
  closures right after each bank) drains into chunk g-1's attention j-loop
  between the scores matmul and AV, hiding the ACT exp latency; all four
  o_proj chunks drain into the long final attention chunk + tail, their
  psums allocated from the then-idle qkv banks (deeper rotation, decoupled
  from the scores banks), with a few fillers reserved for the final
  ones/reciprocal/normalize chain. Fillers
  must NOT drain between a psum write and the emission of its reader
  (3-deep psum pool recycling would clobber it).
"""
import numpy as np
import ml_dtypes
from contextlib import ExitStack
from collections import deque

import concourse.bacc as bacc
import concourse.tile as tile
from concourse import mybir
from concourse.bass_utils import run_bass_kernel_spmd

HIDDEN = 2048
T = 2048
N_HEADS = 16
N_KV = 4
HD = 128
THETA = 500000.0
NCORES = 8
SCALE = HD ** -0.5

F32 = mybir.dt.float32
F32R = mybir.dt.float32r
BF16 = mybir.dt.bfloat16
I32 = mybir.dt.int32

# within-head column permutation: evens then odds (so interleaved rope pairs
# become two contiguous partition halves in feature-major layout)
PERM = np.concatenate([np.arange(0, HD, 2), np.arange(1, HD, 2)])
# pair index p (0..63): p<44: even->pos row 1 (h), odd->row 2 (w); p>=44: row 0 (t)
ROW_MAP = np.array([(1 if p % 2 == 0 else 2) if p < 44 else 0 for p in range(64)])
INVF = (THETA ** (-(np.arange(64, dtype=np.float64) / 64))).astype(np.float32)

NT = T // 128      # 16 token tiles
NG = T // 512      # 4 token chunks
NH_T = HIDDEN // 128  # 16 hidden tiles

# packB (f32r) column layout: ident | ones | mask_l | 4 mask_r blocks
# (rank-factored additive causal mask: mask_l.T @ mask_r[m] = -1e9 where
# q < dk + 128m, applied as a PE matmul accumulation onto the scores psum)
PB_IDENT = 0
PB_ONES = 128
PB_ML = 256
PB_MR = 384
PB_COLS = 384 + 4 * 512


def _build(dbg=False):
    nc = bacc.Bacc("TRN2", target_bir_lowering=False, debug=False)
    d_xT = nc.dram_tensor("xT", [HIDDEN, T], BF16, kind="ExternalInput").ap()
    d_w = nc.dram_tensor("w_slice", [HIDDEN, 512], BF16, kind="ExternalInput").ap()
    d_pa = nc.dram_tensor("packA", [128, T + 2], F32, kind="ExternalInput").ap()
    d_pb = nc.dram_tensor("packB", [128, PB_COLS], F32R, kind="ExternalInput").ap()
    d_pc = nc.dram_tensor("packC", [128, 2 * HIDDEN], BF16, kind="ExternalInput").ap()
    d_yT = nc.dram_tensor("yT", [HIDDEN, T], BF16, kind="ExternalOutput").ap()
    if dbg:
        d_qk = nc.dram_tensor("dbg_qk", [128, 3, T], BF16, kind="ExternalOutput").ap()
        d_cs = nc.dram_tensor("dbg_cs", [128, 2, T], BF16, kind="ExternalOutput").ap()
        d_V = nc.dram_tensor("dbg_V", [128, NT, 128], F32, kind="ExternalOutput").ap()
        d_O = nc.dram_tensor("dbg_O", [128, 2, T], BF16, kind="ExternalOutput").ap()
        d_E = nc.dram_tensor("dbg_E", [128, 8, 512], F32, kind="ExternalOutput").ap()

    TWO_PI = float(2 * np.pi)
    with tile.TileContext(nc) as tc, ExitStack() as ctx:
        const = ctx.enter_context(tc.tile_pool(name="const", bufs=1))
        big = ctx.enter_context(tc.tile_pool(name="big", bufs=1))

        # resident tiles
        w_sb = const.tile([128, NH_T, 512], BF16)       # qkv weight slice
        pa_sb = const.tile([128, T + 2], F32)           # pos | invf | svec
        pb_sb = const.tile([128, PB_COLS], F32R)        # ident | ones | masks
        pc_sb = const.tile([128, 2 * HIDDEN], BF16)     # o_proj rows
        qk_sb = big.tile([128, 3, T], BF16)             # q0|q1|k feature-major
        vf_sb = big.tile([128, T], F32R)                # v feature-major
        V_sb = big.tile([128, NT, 128], F32R)           # V token-major
        O_sb = big.tile([128, 2, T], BF16)              # attention out, feature-major
        cdup = big.tile([128, T], BF16)                 # cos table (dup halves)
        sflip = big.tile([128, T], BF16)                # sin table ([-s; s])

        invf = pa_sb[:, 0:1]
        svec = pa_sb[:, 1:2]
        ident = pb_sb[:, PB_IDENT:PB_IDENT + 128]
        ones = pb_sb[:, PB_ONES:PB_ONES + 128]
        mask_l = pb_sb[:, PB_ML:PB_ML + 128]

        # PSUM budget (8 banks): qkv accum 4 + shared(scores/V-transpose/
        # o_proj/ones) 3 + AV accum 1.
        xtp = ctx.enter_context(tc.tile_pool(name="xt", bufs=8))
        qkvp = ctx.enter_context(tc.tile_pool(name="qkvp", bufs=4, space="PSUM"))
        spp = ctx.enter_context(tc.tile_pool(name="spp", bufs=3, space="PSUM"))
        avp = ctx.enter_context(tc.tile_pool(name="avp", bufs=1, space="PSUM"))
        tbl = ctx.enter_context(tc.tile_pool(name="tbl", bufs=1))
        rp = ctx.enter_context(tc.tile_pool(name="rope", bufs=2))
        ep = ctx.enter_context(tc.tile_pool(name="ep", bufs=6))
        rv = ctx.enter_context(tc.tile_pool(name="rv", bufs=2))
        racc = ctx.enter_context(tc.tile_pool(name="racc", bufs=2))
        yo = ctx.enter_context(tc.tile_pool(name="yo", bufs=6))

        # PE filler queue: closures emitting one PE matmul each, drained
        # between attention-chain matmuls to hide exp latency.
        pe_fill = deque()

        def fill(n):
            for _ in range(min(n, len(pe_fill))):
                pe_fill.popleft()()

        def emit_tables():
            # rope tables, all chunks upfront (keeps Sin/Exp from thrashing
            # the ACT table). invf pre-divided by 2pi on host: turns =
            # pos*invf reduced to [-0.5,0.5] via rne f32->i32 roundtrip; Sin
            # with 2pi (and per-half sign) folded into the ACT scale.
            for gt in range(NG):
                tsl2 = np.s_[512 * gt:512 * (gt + 1)]
                turns = tbl.tile([128, 512], F32, tag="turns", name=f"turns{gt}")
                turns_c = tbl.tile([128, 512], F32, tag="turnsc", name=f"turnsc{gt}")
                tint = tbl.tile([128, 512], I32, tag="ti", name=f"ti{gt}")
                tflt = tbl.tile([128, 512], F32, tag="tf", name=f"tf{gt}")
                # rne roundtrips on ACT (idle during slot 0; keeps Pool free
                # for the v-copy that gates the V transposes)
                nc.vector.tensor_scalar_mul(
                    turns[:], pa_sb[:, 2 + 512 * gt:2 + 512 * (gt + 1)], invf)
                nc.vector.tensor_scalar_add(turns_c[:], turns[:], 0.25)
                nc.scalar.copy(tint[:], turns[:])              # round to nearest
                nc.scalar.copy(tflt[:], tint[:])
                nc.vector.tensor_sub(turns[:], turns[:], tflt[:])
                nc.scalar.activation(sflip[:, tsl2], turns[:],
                                     mybir.ActivationFunctionType.Sin,
                                     bias=0.0, scale=svec)
                # cos path: +0.25 turns offset (cos x = sin(x + pi/2))
                nc.scalar.copy(tint[:], turns_c[:])
                nc.scalar.copy(tflt[:], tint[:])
                nc.vector.tensor_sub(turns_c[:], turns_c[:], tflt[:])
                nc.scalar.activation(cdup[:, tsl2], turns_c[:],
                                     mybir.ActivationFunctionType.Sin,
                                     bias=0.0, scale=TWO_PI)

        def emit_rope(g, t3):
            # rope: o = x*cdup + swap(x)*sflip, swap via DVE cross-partition
            # copies; all bf16 for the 2x DVE rate
            tsl = np.s_[512 * g:512 * (g + 1)]
            x = qk_sb[:, t3, tsl]
            xs = rp.tile([128, 512], BF16, tag="xs", name=f"xs{g}_{t3}")
            t1 = rp.tile([128, 512], BF16, tag="t1", name=f"t1_{g}_{t3}")
            t2 = rp.tile([128, 512], BF16, tag="t2", name=f"t2_{g}_{t3}")
            nc.vector.tensor_copy(xs[0:64, :], x[64:128, :])
            nc.vector.tensor_copy(xs[64:128, :], x[0:64, :])
            nc.vector.tensor_mul(t1[:], x, cdup[:, tsl])
            nc.vector.tensor_mul(t2[:], xs[:], sflip[:, tsl])
            nc.vector.tensor_add(x, t1[:], t2[:])

        xts = {}

        def emit_proj_dmas(g):
            # xT loads for chunk g, issued one slot ahead so the transfers
            # complete before the proj matmuls drain from the fill queue.
            # Chunk 0 is latency-critical: k/v weight columns load first (2
            # halves) to feed phase A, the first xT batch is split, and the
            # q weight columns + rope-table inputs follow.
            tsl = np.s_[512 * g:512 * (g + 1)]
            xts[g] = []
            for hb in range(NH_T // 4):
                xt_b = xtp.tile([128, 4, 512], BF16, tag="xt", name=f"xt_{g}_{hb}")
                # first chunk-0 batch split in half, w/xT interleaved: the
                # first matmul needs only h0-1, so it starts ~1.5us earlier
                halves = ((0, 4),)
                for a0, a1 in halves:
                    if g == 0:
                        nc.sync.dma_start(
                            out=w_sb[:, 4 * hb + a0:4 * hb + a1, :],
                            in_=d_w[512 * hb + 128 * a0:512 * hb + 128 * a1, :]
                            .rearrange("(a p) c -> p a c", p=128))
                    nc.sync.dma_start(
                        out=xt_b[:, a0:a1, :],
                        in_=d_xT[512 * hb + 128 * a0:512 * hb + 128 * a1, tsl]
                        .rearrange("(a p) c -> p a c", p=128))
                xts[g].append(xt_b)
                if g == 0 and hb == 1:
                    # table inputs land after the first two proj transfers
                    nc.sync.dma_start(out=pa_sb, in_=d_pa)
                if g == 1 and hb == 1:
                    # mask/ones land between the chunk-1 loads: early enough
                    # for attn(0)'s first mask matmul, late enough not to
                    # delay the xt batches that gate slot-1's proj fillers
                    nc.sync.dma_start(out=pb_sb[:, 256:PB_COLS],
                                      in_=d_pb[:, 256:PB_COLS])

        def emit_proj_fill(g):
            # qkv projection chunk: qkv[f, t] = w.T @ xT, 4 psum feature
            # groups. Chunk 0 is emitted hb-major (tracks DMA arrival order);
            # later chunks bank-major with v first, so each tensor's psum ->
            # sbuf copy (+rope, +V transposes) runs mid-slot as a queued
            # closure instead of serializing at the slot boundary.
            tsl = np.s_[512 * g:512 * (g + 1)]
            psums = [qkvp.tile([128, 512], F32, tag="qkvps", name=f"qkvps_{g}_{i}")
                     for i in range(4)]

            def mm(h, i):
                nc.tensor.matmul(
                    psums[i][:], w_sb[:, h, 128 * i:128 * (i + 1)],
                    xts[g][h // 4][:, h % 4, :], start=(h == 0), stop=(h == NH_T - 1))

            def copy_rope(i):
                if i == 3:
                    # per-tile v copies on alternating engines so each V
                    # transpose waits only on its own 128-col copy
                    # (Pool/GPSIMD cannot access PSUM)
                    for j in range(4):
                        src = psums[3][:, 128 * j:128 * (j + 1)]
                        dst = vf_sb[:, 512 * g + 128 * j:512 * g + 128 * (j + 1)]
                        if j % 2 == 0:
                            nc.vector.tensor_copy(dst, src)
                        else:
                            nc.scalar.copy(dst, src)
                elif i == 1:
                    nc.scalar.copy(qk_sb[:, i, tsl], psums[i][:])
                    emit_rope(g, i)
                else:
                    nc.vector.tensor_copy(qk_sb[:, i, tsl], psums[i][:])
                    emit_rope(g, i)

            def tr(j):
                pt = spp.tile([128, 512], F32, tag="sp", name=f"vt{j}")
                nc.tensor.transpose(pt[:, 0:128].bitcast(F32R),
                                    vf_sb[:, 128 * j:128 * (j + 1)], ident)
                nc.vector.tensor_copy(V_sb[:, j, :], pt[:, 0:128].bitcast(F32R))

            if g == 0:
                # two phases, hb-major within each (tracks xT DMA arrival):
                # v+k banks first so their copy/rope/V-transpose overlap the
                # q banks' matmuls instead of serializing after them
                for bank_pair in ((3, 2), (0, 1)):
                    for h in range(NH_T):
                        for i in bank_pair:
                            pe_fill.append(lambda h=h, i=i: mm(h, i))
                    for i in bank_pair:
                        pe_fill.append(lambda i=i: copy_rope(i))
                    if bank_pair == (3, 2):
                        for j in range(4):
                            pe_fill.append(lambda j=j: tr(j))
            else:
                for i in (3, 2, 0, 1):
                    for h in range(NH_T):
                        pe_fill.append(lambda h=h, i=i: mm(h, i))
                    pe_fill.append(lambda i=i: copy_rope(i))
                    if i == 3:
                        for j in range(4 * g, 4 * g + 4):
                            pe_fill.append(lambda j=j: tr(j))

        def emit_attn(c, reserve=0):
            # attention for chunk c: h-major; per j: scores (+mask prefill on
            # diagonals) -> exp -> AV accumulate + row-sum chains; then
            # ones-matmul denominator broadcast, reciprocal, normalize.
            # PE fillers drain evenly across the 8(c+1) chain iterations.
            tsl = np.s_[512 * c:512 * (c + 1)]
            jmax = 4 * c + 4
            rate = max(0, len(pe_fill) - reserve) / (2 * jmax)
            acc = [0.0]

            def fill_acc(frac):
                acc[0] += rate * frac
                n = int(acc[0])
                if n:
                    acc[0] -= n
                    fill(n)

            for h in range(2):
                po = avp.tile([128, 512], F32, tag="av", name=f"po{c}_{h}")
                ra = racc.tile([128, 512], F32R, tag="ra", name=f"ra{c}_{h}")
                rb = racc.tile([128, 512], F32R, tag="rb", name=f"rb{c}_{h}")
                for j in range(jmax):
                    m = j - 4 * c
                    # diagonal narrowing; m=3 widened to 256 cols so its
                    # scores/mask/AV matmuls stay at the 1-cycle/row rate
                    # (<256 free dim costs 4x on fp32r): the extra columns
                    # are fully masked (-1e9 -> exp gives exact zeros)
                    r0 = (0, 128, 256, 256)[max(m, 0)]
                    ps = spp.tile([128, 512], F32, tag="sp", name=f"s{c}_{h}_{j}")
                    nc.tensor.matmul(ps[:, r0:512],
                                     qk_sb[:, 2, 128 * j:128 * (j + 1)],
                                     qk_sb[:, h, 512 * c + r0:512 * (c + 1)],
                                     start=True, stop=(m < 0))
                    if m >= 0:
                        # additive causal mask via rank-factored matmul
                        # accumulated onto the scores psum. mask_r[m] is
                        # provably zero beyond the staircase boundary, so
                        # m=0/m=1 only need 256 columns (m=2/3 already are
                        # 256 wide; narrower would hit the 4-cycles/row
                        # penalty)
                        r1 = r0 + 256 if m < 2 else 512
                        nc.tensor.matmul(
                            ps[:, r0:r1], mask_l,
                            pb_sb[:, PB_MR + 512 * m + r0:PB_MR + 512 * m + r1],
                            start=False, stop=True, skip_group_check=True)
                    # exp MUST be emitted before draining fillers: a filler
                    # allocating from the 3-deep scores psum pool between the
                    # scores write and its exp read would recycle the bank
                    # and clobber the scores
                    E = ep.tile([128, 512], F32R, tag="e", name=f"e{c}_{h}_{j}")
                    nc.scalar.activation(E[:, r0:512], ps[:, r0:512],
                                         mybir.ActivationFunctionType.Exp,
                                         scale=SCALE)
                    if dbg and h == 0 and c <= 1 and m >= 0:
                        nc.sync.dma_start(out=d_E[:, 4 * c + m, :],
                                          in_=E[:].bitcast(F32))
                    fill_acc(1.0)
                    nc.tensor.matmul(po[:, r0:512], V_sb[:, j, :], E[:, r0:512],
                                     start=(j == 0), stop=(j == jmax - 1),
                                     skip_group_check=True)
                    fill_acc(0.0)
                    # row-sum partial chains: ra on DVE (odd j + the last),
                    # rb on Pool; merged in PSUM by the ones-matmul pair
                    if j == 0:
                        nc.vector.tensor_copy(ra[:], E[:])
                    elif j == 1:
                        if r0 > 0:
                            nc.gpsimd.memset(rb[:, 0:r0].bitcast(F32), 0.0)
                        nc.gpsimd.tensor_copy(rb[:, r0:512], E[:, r0:512])
                    elif j % 2 == 1:
                        nc.vector.tensor_add(ra[:, r0:512], ra[:, r0:512],
                                             E[:, r0:512])
                    else:
                        nc.gpsimd.tensor_add(rb[:, r0:512], rb[:, r0:512],
                                             E[:, r0:512])
                # denominator broadcast across partitions: two accumulating
                # ones-matmuls merge the two chains in PSUM
                pr = spp.tile([128, 512], F32, tag="sp", name=f"pr{c}_{h}")
                nc.tensor.matmul(pr[:], ones, rb[:], start=True, stop=False)
                nc.tensor.matmul(pr[:], ones, ra[:], start=False, stop=True)
                rinv = rv.tile([128, 512], F32, tag="rv", name=f"rinv{c}_{h}")
                nc.vector.reciprocal(rinv[:], pr[:])
                nc.vector.tensor_mul(O_sb[:, h, tsl], po[:], rinv[:])
                fill(2)

        def emit_oproj_fill(c):
            # o_proj partial chunk: yT[:, tsl] = sum_h wo_h.T @ O_h; queued
            # as PE filler. yt copies rotate across DVE/ACT/Pool; batched
            # stores, with the last chunk tapering to 2-tile stores (and
            # fast-engine copies) to shrink the drain tail.
            tsl = np.s_[512 * c:512 * (c + 1)]
            groups = [2] * 8
            stage = {}
            base = {}
            i0 = 0
            for gi, n in enumerate(groups):
                for i in range(i0, i0 + n):
                    base[i] = (gi, i - i0, n, i0)
                i0 += n

            def do_tile(i):
                gi, off, n, istart = base[i]
                if off == 0:
                    stage[gi] = yo.tile([128, n, 512], BF16, tag="yo",
                                        name=f"yo{c}_{gi}")
                # all o_proj drains in the epilogue, where the 4 qkv psum
                # banks are idle: deeper rotation than spp and decoupled
                # from the attention scores banks
                py = qkvp.tile([128, 512], F32, tag="qkvps", name=f"y{c}_{i}")
                for h in range(2):
                    nc.tensor.matmul(py[:],
                                     pc_sb[:, 2048 * h + 128 * i:2048 * h + 128 * (i + 1)],
                                     O_sb[:, h, tsl], start=(h == 0), stop=(h == 1))
                st = stage[gi]
                # py is PSUM: only DVE/ACT may read it
                if i % 2 == 0:
                    nc.vector.tensor_copy(st[:, off, :], py[:])
                else:
                    nc.scalar.copy(st[:, off, :], py[:])
                if off == n - 1:
                    nc.sync.dma_start(
                        out=d_yT[128 * istart:128 * (istart + n), tsl].rearrange(
                            "(a p) c -> p a c", p=128),
                        in_=st)

            for i in range(NH_T):
                pe_fill.append(lambda i=i: do_tile(i))

        # ---- schedule ----
        # slot 0: proj(0) + tables. DMA order: chunk-0 loads (kv-first),
        # ident/masks (needed by attn(0)), late rope-table inputs, chunk-1
        # loads, o_proj weights (needed slot 2)
        emit_proj_dmas(0)
        emit_proj_fill(0)
        emit_tables()
        nc.sync.dma_start(out=pb_sb[:, 0:256], in_=d_pb[:, 0:256])
        emit_proj_dmas(1)
        nc.sync.dma_start(out=pc_sb, in_=d_pc)
        fill(len(pe_fill))
        # slots 1..3: attn(g-1) interleaved with proj(g) [+ oproj(g-3)]
        for g in range(1, NG):
            emit_proj_fill(g)
            if g + 1 < NG:
                emit_proj_dmas(g + 1)
            emit_attn(g - 1)
            fill(len(pe_fill))
        # epilogue: attn(3) with oproj(1)+oproj(2) as filler, then oproj(3)
        emit_oproj_fill(NG - 4)
        emit_oproj_fill(NG - 3)
        emit_oproj_fill(NG - 2)
        emit_attn(NG - 1, reserve=10)
        emit_oproj_fill(NG - 1)
        fill(len(pe_fill))

        if dbg:
            nc.sync.dma_start(out=d_qk, in_=qk_sb[:])
            nc.sync.dma_start(out=d_cs[:, 0, :], in_=cdup[:])
            nc.sync.dma_start(out=d_cs[:, 1, :], in_=sflip[:])
            nc.sync.dma_start(out=d_V, in_=V_sb[:].bitcast(F32))
            nc.sync.dma_start(out=d_O, in_=O_sb[:])

    nc.compile()
    return nc


_NC_CACHE = None


def _get_nc():
    global _NC_CACHE
    if _NC_CACHE is None:
        _NC_CACHE = _build()
    return _NC_CACHE


def _host_prep(positions, hidden_states, w_qkv, w_o):
    positions = np.asarray(positions, dtype=np.int32)
    hidden_states = np.asarray(hidden_states, dtype=np.float32)
    w_qkv = np.asarray(w_qkv, dtype=np.float32)
    w_o = np.asarray(w_o, dtype=np.float32)
    bf = ml_dtypes.bfloat16

    xT = np.ascontiguousarray(hidden_states.T).astype(bf)

    # packA: invf/2pi | svec (+-2pi) | pos rows (f32-exact ints)
    packA = np.empty((128, T + 2), dtype=np.float32)
    packA[:, 0] = np.concatenate([INVF, INVF]) / (2 * np.pi)
    tp = np.float32(2 * np.pi)
    packA[:, 1] = np.concatenate([-tp * np.ones(64, np.float32),
                                  tp * np.ones(64, np.float32)])
    packA[:, 2:] = positions[np.concatenate([ROW_MAP, ROW_MAP])].astype(np.float32)

    # packB: ident | ones | rank-factored causal mask (baseline-proven):
    #   mask_l[p, dk] = [p <= dk],  mask_r[c, m, q] = -1e9 at c = max(q -
    #   128m + 1, 0) (c <= 127) => (mask_l.T @ mask_r[m])[dk, q] = -1e9
    #   iff q < dk + 128m
    packB = np.zeros((128, PB_COLS), dtype=np.float32)
    packB[:, PB_IDENT:PB_IDENT + 128] = np.eye(128, dtype=np.float32)
    packB[:, PB_ONES:PB_ONES + 128] = 1.0
    packB[:, PB_ML:PB_ML + 128] = (
        np.arange(128)[:, None] <= np.arange(128)[None, :]).astype(np.float32)
    for m in range(4):
        c = np.maximum(np.arange(512) - 128 * m + 1, 0)
        valid = c <= 127
        blk = np.zeros((128, 512), dtype=np.float32)
        blk[c[valid], np.arange(512)[valid]] = -1e9
        packB[:, PB_MR + 512 * m:PB_MR + 512 * (m + 1)] = blk
    dq = np.arange(512)[None, :]
    dk = np.arange(128)[:, None]
    for m in range(4):
        got = packB[:, PB_ML:PB_ML + 128].T @ packB[:, PB_MR + 512 * m:PB_MR + 512 * (m + 1)]
        want = np.where(dq < dk + 128 * m, np.float32(-1e9), np.float32(0.0))
        assert np.array_equal(got, want), f"mask factorization wrong for m={m}"

    q_size = N_HEADS * HD
    kv_size = N_KV * HD
    in_maps = []
    for c in range(NCORES):
        cols = [w_qkv[:, 2 * c * HD + PERM], w_qkv[:, (2 * c + 1) * HD + PERM]]
        kc = c // 2
        cols.append(w_qkv[:, q_size + kc * HD + PERM])
        cols.append(w_qkv[:, q_size + kv_size + kc * HD:q_size + kv_size + (kc + 1) * HD])
        w_slice = np.ascontiguousarray(np.concatenate(cols, axis=1)).astype(bf)
        packC = np.ascontiguousarray(
            w_o[2 * c * HD:(2 * c + 2) * HD].reshape(2, 128, HIDDEN)
            .transpose(1, 0, 2).reshape(128, 2 * HIDDEN)).astype(bf)
        in_maps.append({
            "xT": xT, "w_slice": w_slice, "packA": packA, "packB": packB,
            "packC": packC,
        })
    return in_maps


def kernel(positions, hidden_states, w_qkv, w_o):
    nc = _get_nc()
    in_maps = _host_prep(positions, hidden_states, w_qkv, w_o)
    # one retry: transient NRT/device errors were observed to succeed on
    # re-dispatch
    try:
        res = run_bass_kernel_spmd(nc, in_maps, core_ids=list(range(NCORES)))
    except Exception:
        import time
        time.sleep(2.0)
        res = run_bass_kernel_spmd(nc, in_maps, core_ids=list(range(NCORES)))
    yT = np.zeros((HIDDEN, T), dtype=np.float64)
    for c in range(NCORES):
        yT += res.results[c]["yT"].astype(np.float32)
    return np.ascontiguousarray(yT.T).astype(np.float32)
